# revision 1
# baseline (speedup 1.0000x reference)
"""Local multi-head attention (window=128, look 1/1) on 8 trn2 cores.

Sequence-parallel: core c owns 256 positions (2 windows); k/v halos are
recomputed from zero-padded input slices, so no collectives are needed.
Edge-window masking is exact: padded keys contribute exp(0)=1 to the
softmax denominator, which is subtracted via a per-core correction, and
zero v rows contribute nothing to the numerator.

Layouts per core (n = b*Sc + s_local, b-major):
  xqT (E, B*256), xkT/xvT (E, B*512) halo, weights pre-transposed (W.T).
  q/k/v projections run in float32r (full PE rate at N=512); q/k are
  kept as bf16 in (feature, position) layout, v as bf16 in (position,
  feature) layout so QK^T, softmax, and attn@V all avoid transposes
  except a PE transpose of the attention probabilities. The output
  projection runs in bf16.
"""

import sys

sys.path.insert(0, "/opt/trn_rl_repo")

import ml_dtypes
import numpy as np

import concourse.bass as bass
import concourse.mybir as mybir
from concourse import bacc
from concourse.bass_utils import run_bass_kernel_spmd
from concourse.masks import make_identity
from concourse.tile import TileContext

S, B, E, H, HD, W = 2048, 4, 1024, 16, 64, 128
NC = 8
SC = S // NC          # 256 positions per core
SH = SC + 2 * W       # 512 halo positions
NQ = B * SC           # 1024 query rows
NKV = B * SH          # 2048 k/v rows
F32 = mybir.dt.float32
F32R = mybir.dt.float32r
BF16 = mybir.dt.bfloat16

_COMPILED = {}


def _build_nc():
    nc = bacc.Bacc("TRN2", target_bir_lowering=False, debug=False, num_devices=NC)
    xq = nc.dram_tensor("xq", [E, NQ], BF16, kind="ExternalInput").ap()
    xk = nc.dram_tensor("xk", [E, NKV], BF16, kind="ExternalInput").ap()
    xv = nc.dram_tensor("xv", [E, NKV], BF16, kind="ExternalInput").ap()
    wq = nc.dram_tensor("wq", [E, E], BF16, kind="ExternalInput").ap()
    wk = nc.dram_tensor("wk", [E, E], BF16, kind="ExternalInput").ap()
    wv = nc.dram_tensor("wv", [E, E], BF16, kind="ExternalInput").ap()
    wo = nc.dram_tensor("wo", [E, E], BF16, kind="ExternalInput").ap()
    bo = nc.dram_tensor("bo", [E // 128, 128, 1], F32, kind="ExternalInput").ap()
    inv = nc.dram_tensor("inv", [2, 128, 1], F32, kind="ExternalInput").ap()
    out = nc.dram_tensor("out", [E, NQ], F32, kind="ExternalOutput").ap()

    KT = E // 128  # 8 contraction tiles

    with TileContext(nc) as tc:
        with (
            tc.tile_pool(name="persist", bufs=1) as pp,
            tc.tile_pool(name="psum_proj", bufs=2, space="PSUM") as ppj,
        ):
            ident = pp.tile([128, 128], BF16, name="ident", tag="ident")
            make_identity(nc, ident[:])
            inv_sb = [
                pp.tile([128, 1], F32, name=f"inv{w}", tag=f"inv{w}")
                for w in range(2)
            ]
            for w in range(2):
                nc.sync.dma_start(out=inv_sb[w][:], in_=inv[w])
            bo_sb = [
                pp.tile([128, 1], F32, name=f"bo{g}", tag=f"bo{g}")
                for g in range(KT)
            ]
            for g in range(KT):
                nc.sync.dma_start(out=bo_sb[g][:], in_=bo[g])

            qT = [
                pp.tile([128, NQ], BF16, name=f"qT{m}", tag=f"qT{m}")
                for m in range(KT)
            ]
            kT = [
                pp.tile([128, NKV], BF16, name=f"kT{m}", tag=f"kT{m}")
                for m in range(KT)
            ]
            vb = [
                pp.tile([128, E], BF16, name=f"v{m}", tag=f"v{m}")
                for m in range(NKV // 128)
            ]
            aoT = [
                pp.tile([128, NQ], BF16, name=f"aoT{m}", tag=f"aoT{m}")
                for m in range(KT)
            ]

            # shared streaming pool for all three input projections:
            # w0..w7 hold the (transposed) weight, x0..x7 hold a
            # 1024-column chunk of the transposed input.
            with tc.tile_pool(name="proj", bufs=1) as wp:

                def proj_phase(w_dram, x_dram, n_cols, out_feature_major):
                    w_sb = [
                        wp.tile([128, E], BF16, name=f"w{k}", tag=f"w{k}")
                        for k in range(KT)
                    ]
                    for k in range(KT):
                        nc.sync.dma_start(out=w_sb[k][:], in_=w_dram[bass.ts(k, 128), :])
                    for half in range(n_cols // 1024):
                        x_sb = [
                            wp.tile([128, 1024], BF16, name=f"x{k}", tag=f"x{k}")
                            for k in range(KT)
                        ]
                        for k in range(KT):
                            nc.sync.dma_start(
                                out=x_sb[k][:],
                                in_=x_dram[bass.ts(k, 128), bass.ts(half, 1024)],
                            )
                        if out_feature_major:
                            # out[f, n] = sum_e W.T[e,f] x[e,n]
                            for m in range(KT):
                                for n in range(2):
                                    ps = ppj.tile([128, 512], F32, name="ps", tag="ps")
                                    for k in range(KT):
                                        nc.tensor.matmul(
                                            ps[:],
                                            w_sb[k][:, bass.ts(m, 128)],
                                            x_sb[k][:, bass.ts(n, 512)],
                                            start=(k == 0),
                                            stop=(k == KT - 1),
                                        )
                                    yield ps, m, half * 1024 + n * 512
                        else:
                            # out[n, f] = sum_e x[e,n] W.T[e,f]
                            for m in range(8):
                                for n in range(2):
                                    ps = ppj.tile([128, 512], F32, name="ps", tag="ps")
                                    for k in range(KT):
                                        nc.tensor.matmul(
                                            ps[:],
                                            x_sb[k][:, bass.ts(m, 128)],
                                            w_sb[k][:, bass.ts(n, 512)],
                                            start=(k == 0),
                                            stop=(k == KT - 1),
                                        )
                                    yield ps, half * 8 + m, n * 512

                for ps, m, col in proj_phase(wq, xq, NQ, True):
                    nc.vector.tensor_copy(qT[m][:, col : col + 512], ps[:])
                for ps, m, col in proj_phase(wk, xk, NKV, True):
                    nc.vector.tensor_copy(kT[m][:, col : col + 512], ps[:])
                for ps, m, col in proj_phase(wv, xv, NKV, False):
                    nc.vector.tensor_copy(vb[m][:, col : col + 512], ps[:])

            # ---- attention ----
            with (
                tc.tile_pool(name="attn", bufs=3) as ap,
                tc.tile_pool(name="ps_s", bufs=2, space="PSUM") as ps_s,
                tc.tile_pool(name="ps_t", bufs=2, space="PSUM") as ps_t,
                tc.tile_pool(name="ps_o", bufs=2, space="PSUM") as ps_o,
            ):
                for b in range(B):
                    for wl in range(2):
                        for h in range(H):
                            t, r0 = h // 2, (h % 2) * 64
                            qcol = b * SC + wl * 128
                            kcol = b * SH + wl * 128
                            scores = ps_s.tile(
                                [128, 384], F32, name="scores", tag="scores"
                            )
                            nc.tensor.matmul(
                                scores[:],
                                qT[t][r0 : r0 + 64, qcol : qcol + 128],
                                kT[t][r0 : r0 + 64, kcol : kcol + 384],
                                start=True,
                                stop=True,
                            )
                            esc = ap.tile([128, 384], BF16, name="esc", tag="esc")
                            den = ap.tile([128, 1], F32, name="den", tag="den")
                            nc.scalar.activation(
                                esc[:],
                                scores[:],
                                mybir.ActivationFunctionType.Exp,
                                scale=float(HD) ** -0.5,
                                accum_out=den[:],
                            )
                            rec = ap.tile([128, 1], F32, name="rec", tag="rec")
                            nc.vector.tensor_add(rec[:], den[:], inv_sb[wl][:])
                            nc.vector.reciprocal(rec[:], rec[:])
                            attn = ap.tile([128, 384], BF16, name="attn", tag="attn")
                            nc.vector.tensor_scalar_mul(attn[:], esc[:], rec[:])
                            attnT = ap.tile(
                                [128, 384], BF16, name="attnT", tag="attnT"
                            )
                            for u in range(3):
                                pt = ps_t.tile([128, 128], BF16, name="pt", tag="pt")
                                nc.tensor.transpose(
                                    pt[:], attn[:, bass.ts(u, 128)], ident[:]
                                )
                                nc.vector.tensor_copy(attnT[:, bass.ts(u, 128)], pt[:])
                            po = ps_o.tile([64, 128], F32, name="po", tag="po")
                            for u in range(3):
                                nc.tensor.matmul(
                                    po[:],
                                    vb[b * 4 + wl + u][:, h * 64 : (h + 1) * 64],
                                    attnT[:, bass.ts(u, 128)],
                                    start=(u == 0),
                                    stop=(u == 2),
                                )
                            nc.scalar.copy(
                                aoT[t][r0 : r0 + 64, qcol : qcol + 128], po[:]
                            )

            # ---- output projection (bf16) + bias ----
            with (
                tc.tile_pool(name="oproj", bufs=1) as wpo,
                tc.tile_pool(name="ostage", bufs=2) as op,
            ):
                wo_sb = [
                    wpo.tile([128, E], BF16, name=f"wo{k}", tag=f"wo{k}")
                    for k in range(KT)
                ]
                for k in range(KT):
                    nc.sync.dma_start(out=wo_sb[k][:], in_=wo[bass.ts(k, 128), :])
                for m in range(KT):
                    ot = op.tile([128, NQ], F32, name="ot", tag="ot")
                    for n in range(NQ // 512):
                        ps = ppj.tile([128, 512], F32, name="ps", tag="ps")
                        for k in range(KT):
                            nc.tensor.matmul(
                                ps[:],
                                wo_sb[k][:, bass.ts(m, 128)],
                                aoT[k][:, bass.ts(n, 512)],
                                start=(k == 0),
                                stop=(k == KT - 1),
                            )
                        nc.scalar.activation(
                            ot[:, bass.ts(n, 512)],
                            ps[:],
                            mybir.ActivationFunctionType.Identity,
                            bias=bo_sb[m][:],
                        )
                    nc.sync.dma_start(out=out[bass.ts(m, 128), :], in_=ot[:])

    nc.finalize()
    return nc


def _shard_inputs(query, key, value, Wq, Wk, Wv, Wo, bo, key_padding_mask):
    del key_padding_mask  # all-False in this problem; handled exactly by design
    q = np.asarray(query, np.float32)
    k = np.asarray(key, np.float32)
    v = np.asarray(value, np.float32)
    wqT = np.ascontiguousarray(np.asarray(Wq, np.float32).T.astype(ml_dtypes.bfloat16))
    wkT = np.ascontiguousarray(np.asarray(Wk, np.float32).T.astype(ml_dtypes.bfloat16))
    wvT = np.ascontiguousarray(np.asarray(Wv, np.float32).T.astype(ml_dtypes.bfloat16))
    woT = np.ascontiguousarray(
        np.asarray(Wo, np.float32).T.astype(ml_dtypes.bfloat16)
    )
    bo_r = np.ascontiguousarray(np.asarray(bo, np.float32).reshape(E // 128, 128, 1))
    in_maps = []
    for c in range(NC):
        s0 = c * SC
        xqT = np.ascontiguousarray(q[s0 : s0 + SC].transpose(2, 1, 0).reshape(E, NQ).astype(ml_dtypes.bfloat16))
        kh = np.zeros((SH, B, E), np.float32)
        vh = np.zeros((SH, B, E), np.float32)
        lo, hi = s0 - W, s0 + SC + W
        glo, ghi = max(lo, 0), min(hi, S)
        kh[glo - lo : ghi - lo] = k[glo:ghi]
        vh[glo - lo : ghi - lo] = v[glo:ghi]
        xkT = np.ascontiguousarray(kh.transpose(2, 1, 0).reshape(E, NKV).astype(ml_dtypes.bfloat16))
        xvT = np.ascontiguousarray(vh.transpose(2, 1, 0).reshape(E, NKV).astype(ml_dtypes.bfloat16))
        inv_c = np.zeros((2, 128, 1), np.float32)
        if c == 0:
            inv_c[0] = -float(W)
        if c == NC - 1:
            inv_c[1] = -float(W)
        in_maps.append(
            {
                "xq": xqT, "xk": xkT, "xv": xvT,
                "wq": wqT, "wk": wkT, "wv": wvT, "wo": woT,
                "bo": bo_r, "inv": inv_c,
            }
        )
    return in_maps


def kernel(**inputs) -> np.ndarray:
    if "nc" not in _COMPILED:
        _COMPILED["nc"] = _build_nc()
    nc = _COMPILED["nc"]
    in_maps = _shard_inputs(**inputs)
    res = run_bass_kernel_spmd(nc, in_maps, list(range(NC)))
    out = np.empty((S, B, E), np.float32)
    for c in range(NC):
        outT = res.results[c]["out"]  # (E, B*SC)
        out[c * SC : (c + 1) * SC] = outT.reshape(E, B, SC).transpose(2, 1, 0)
    return out



# revision 5
# speedup vs baseline: 1.7096x; 1.7096x over previous
"""Local multi-head attention (window=128, look 1/1) on 8 trn2 cores.

Sharding: core c owns (batch b = c//2, sequence half h = c%2) = 1024 query
positions; k/v are recomputed over a 1280-position halo slice (one extra
128-window each side, zero-padded at the global sequence edges).

Attention runs in transposed-score orientation to avoid PE transposes:
  scoresT[j, i] = k_j . q_i  per (head, key-window u), with the query block
  being the up-to-3 query windows that attend key-window u. exp() runs on
  the scalar engine (PSUM -> SBUF bf16); softmax denominators come from
  col-tiled ones-matmuls into the same partition rows as the output
  accumulator, so the normalize is two aligned vector ops. Zero-padded
  edge key-windows would contribute exp(0)=1 per fake key, so their
  exp-scores are multiplied by a per-core 0/1 mask (exactly reproducing
  the reference's -inf masking of out-of-range positions).

Head pairs (2t, 2t+1) share feature tile t: scores matmuls are row-tiled
(K=64 at array rows 0:64 / 64:128), attention@V and denominator matmuls
are col-tiled (M=64 writing partitions 0:64 / 64:128), so both heads run
concurrently in the PE array.
"""

import sys

sys.path.insert(0, "/opt/trn_rl_repo")

import ml_dtypes
import numpy as np

import concourse.bass as bass
import concourse.mybir as mybir
from concourse import bacc
from concourse.bass_utils import run_bass_kernel_spmd
from concourse.tile import TileContext

S, B, E, H, HD, W = 2048, 4, 1024, 16, 64, 128
NC = 8
SC = S // 2           # 1024 positions per core (one batch, half the sequence)
SH = SC + 2 * W       # 1280 halo positions = 10 key windows
NW = SC // W          # 8 owned query windows
NKW = SH // W         # 10 key windows (halo coords); owned = 1..8
KT = E // 128         # 8 contraction tiles
F32 = mybir.dt.float32
BF16 = mybir.dt.bfloat16

_COMPILED = {}


def _qblock(u):
    """Query-window range (halo coords, clipped to owned 1..NW) attending
    key-window u, as column range [qc0, qc1) of the 1024 owned queries."""
    wlo, whi = max(u - 1, 1), min(u + 1, NW)
    return (wlo - 1) * W, whi * W


def _build_nc():
    nc = bacc.Bacc("TRN2", target_bir_lowering=False, debug=False, num_devices=NC)
    xq = nc.dram_tensor("xq", [E, SC], BF16, kind="ExternalInput").ap()
    xk = nc.dram_tensor("xk", [E, SH], BF16, kind="ExternalInput").ap()
    xv = nc.dram_tensor("xv", [E, SH], BF16, kind="ExternalInput").ap()
    wq = nc.dram_tensor("wq", [E, E], BF16, kind="ExternalInput").ap()
    wk = nc.dram_tensor("wk", [E, E], BF16, kind="ExternalInput").ap()
    wv = nc.dram_tensor("wv", [E, E], BF16, kind="ExternalInput").ap()
    wo = nc.dram_tensor("wo", [E, E], BF16, kind="ExternalInput").ap()
    bo = nc.dram_tensor("bo", [KT, 128, 1], F32, kind="ExternalInput").ap()
    maskL = nc.dram_tensor("maskL", [128, 1], F32, kind="ExternalInput").ap()
    maskR = nc.dram_tensor("maskR", [128, 1], F32, kind="ExternalInput").ap()
    out = nc.dram_tensor("out", [E, SC], F32, kind="ExternalOutput").ap()

    with TileContext(nc) as tc:
        with tc.tile_pool(name="persist", bufs=1) as pp:
            bo_sb = [pp.tile([128, 1], F32, name=f"bo{g}", tag=f"bo{g}") for g in range(KT)]
            for g in range(KT):
                nc.sync.dma_start(out=bo_sb[g][:], in_=bo[g])
            mL = pp.tile([128, 1], F32, name="mL", tag="mL")
            mR = pp.tile([128, 1], F32, name="mR", tag="mR")
            nc.sync.dma_start(out=mL[:], in_=maskL)
            nc.sync.dma_start(out=mR[:], in_=maskR)
            ones = pp.tile([128, 64], BF16, name="ones", tag="ones")
            nc.vector.memset(ones[:], 1.0)

            qT = [pp.tile([128, SC], BF16, name=f"qT{m}", tag=f"qT{m}") for m in range(KT)]
            kT = [pp.tile([128, SH], BF16, name=f"kT{m}", tag=f"kT{m}") for m in range(KT)]
            vb = [pp.tile([128, E], BF16, name=f"v{m}", tag=f"v{m}") for m in range(NKW)]
            aoT = [pp.tile([128, SC], BF16, name=f"aoT{m}", tag=f"aoT{m}") for m in range(KT)]

            # weights + transposed-input staging, double-buffered across phases
            with tc.tile_pool(name="wx", bufs=2) as wx:

                def load_w(w_dram):
                    w_sb = [wx.tile([128, E], BF16, name=f"w{k}", tag=f"w{k}") for k in range(KT)]
                    for k in range(KT):
                        nc.sync.dma_start(out=w_sb[k][:], in_=w_dram[bass.ts(k, 128), :])
                    return w_sb

                def load_x(x_dram, n_cols):
                    x_sb = [wx.tile([128, SH], BF16, name=f"x{k}", tag=f"x{k}") for k in range(KT)]
                    for k in range(KT):
                        nc.sync.dma_start(
                            out=x_sb[k][:, 0:n_cols], in_=x_dram[bass.ts(k, 128), :]
                        )
                    return x_sb

                with tc.tile_pool(name="ppj", bufs=2, space="PSUM") as ppj:
                    # ---- k projection (feature-major: kT[f_tile][128, 1280]) ----
                    w_sb = load_w(wk)
                    x_sb = load_x(xk, SH)
                    for m in range(KT):
                        for c0, c1 in ((0, 512), (512, 1024), (1024, 1280)):
                            ps = ppj.tile([128, 512], F32, name="ps", tag="ps")
                            for k in range(KT):
                                nc.tensor.matmul(
                                    ps[:, 0 : c1 - c0],
                                    w_sb[k][:, bass.ts(m, 128)],
                                    x_sb[k][:, c0:c1],
                                    start=(k == 0),
                                    stop=(k == KT - 1),
                                )
                            nc.vector.tensor_copy(kT[m][:, c0:c1], ps[:, 0 : c1 - c0])

                    # ---- v projection (position-major: vb[pos_chunk][128, 1024]) ----
                    w_sb = load_w(wv)
                    x_sb = load_x(xv, SH)
                    for m in range(NKW):
                        for n in range(2):
                            ps = ppj.tile([128, 512], F32, name="ps", tag="ps")
                            for k in range(KT):
                                nc.tensor.matmul(
                                    ps[:],
                                    x_sb[k][:, bass.ts(m, 128)],
                                    w_sb[k][:, bass.ts(n, 512)],
                                    start=(k == 0),
                                    stop=(k == KT - 1),
                                )
                            nc.scalar.copy(vb[m][:, bass.ts(n, 512)], ps[:])

                    # ---- q projection (feature-major: qT[f_tile][128, 1024]) ----
                    w_sb = load_w(wq)
                    x_sb = load_x(xq, SC)
                    for m in range(KT):
                        for n in range(2):
                            ps = ppj.tile([128, 512], F32, name="ps", tag="ps")
                            for k in range(KT):
                                nc.tensor.matmul(
                                    ps[:],
                                    w_sb[k][:, bass.ts(m, 128)],
                                    x_sb[k][:, bass.ts(n, 512)],
                                    start=(k == 0),
                                    stop=(k == KT - 1),
                                )
                            nc.vector.tensor_copy(qT[m][:, bass.ts(n, 512)], ps[:])

                # prefetch wo during attention
                wo_sb = load_w(wo)

                # ---- attention ----
                with (
                    tc.tile_pool(name="scp", bufs=4, space="PSUM") as scp,
                    tc.tile_pool(name="pop", bufs=2, space="PSUM") as pop,
                    tc.tile_pool(name="dnp", bufs=2, space="PSUM") as dnp,
                    tc.tile_pool(name="esc", bufs=24) as esc,
                    tc.tile_pool(name="rcp", bufs=2) as rcp,
                ):
                    for t in range(KT):
                        et = {}
                        for u in range(NKW):
                            qc0, qc1 = _qblock(u)
                            n = qc1 - qc0
                            eu = []
                            for hh, r0 in ((0, 0), (1, 64)):
                                sc = scp.tile([128, 384], F32, name="sc", tag="sc")
                                nc.tensor.matmul(
                                    sc[:, 0:n],
                                    kT[t][r0 : r0 + 64, bass.ts(u, 128)],
                                    qT[t][r0 : r0 + 64, qc0:qc1],
                                    start=True,
                                    stop=True,
                                )
                                ee = esc.tile([128, 384], BF16, name="ee", tag="ee")
                                nc.scalar.activation(
                                    ee[:, 0:n],
                                    sc[:, 0:n],
                                    mybir.ActivationFunctionType.Exp,
                                    scale=float(HD) ** -0.5,
                                )
                                if u == 0:
                                    nc.vector.tensor_scalar_mul(ee[:, 0:n], ee[:, 0:n], mL[:])
                                elif u == NKW - 1:
                                    nc.vector.tensor_scalar_mul(ee[:, 0:n], ee[:, 0:n], mR[:])
                                eu.append(ee)
                            et[u] = (eu[0], eu[1], qc0, qc1)

                        for qh in range(2):
                            g0, g1 = qh * 512, qh * 512 + 512
                            us = [
                                u for u in range(NKW)
                                if et[u][2] < g1 and et[u][3] > g0
                            ]
                            po = pop.tile([128, 512], F32, name="po", tag="po")
                            den = dnp.tile([128, 512], F32, name="den", tag="den")
                            for i, u in enumerate(us):
                                eA, eB, qc0, qc1 = et[u]
                                c0, c1 = max(qc0, g0), min(qc1, g1)
                                start, stop = (i == 0), (i == len(us) - 1)
                                for hh, (r0, ee) in enumerate(((0, eA), (64, eB))):
                                    rhs = ee[:, c0 - qc0 : c1 - qc0]
                                    nc.tensor.matmul(
                                        po[r0 : r0 + 64, c0 - g0 : c1 - g0],
                                        vb[u][:, (2 * t + hh) * 64 : (2 * t + hh + 1) * 64],
                                        rhs,
                                        start=start,
                                        stop=stop,
                                    )
                                    nc.tensor.matmul(
                                        den[r0 : r0 + 64, c0 - g0 : c1 - g0],
                                        ones[:, 0:64],
                                        rhs,
                                        start=start,
                                        stop=stop,
                                    )
                            rec = rcp.tile([128, 512], F32, name="rec", tag="rec")
                            nc.vector.reciprocal(rec[:], den[:])
                            nc.vector.tensor_mul(aoT[t][:, g0:g1], po[:], rec[:])

                # ---- output projection + bias ----
                with (
                    tc.tile_pool(name="ppo", bufs=2, space="PSUM") as ppo,
                    tc.tile_pool(name="ot", bufs=2) as otp,
                ):
                    for m in range(KT):
                        ot = otp.tile([128, SC], F32, name="ot", tag="ot")
                        for n in range(2):
                            ps = ppo.tile([128, 512], F32, name="ps", tag="ps")
                            for k in range(KT):
                                nc.tensor.matmul(
                                    ps[:],
                                    wo_sb[k][:, bass.ts(m, 128)],
                                    aoT[k][:, bass.ts(n, 512)],
                                    start=(k == 0),
                                    stop=(k == KT - 1),
                                )
                            nc.scalar.activation(
                                ot[:, bass.ts(n, 512)],
                                ps[:],
                                mybir.ActivationFunctionType.Identity,
                                bias=bo_sb[m][:],
                            )
                        nc.sync.dma_start(out=out[bass.ts(m, 128), :], in_=ot[:])

    nc.finalize()
    return nc


def _shard_inputs(query, key, value, Wq, Wk, Wv, Wo, bo, key_padding_mask):
    del key_padding_mask  # all-False in this problem; exact by construction
    q = np.asarray(query, np.float32)
    k = np.asarray(key, np.float32)
    v = np.asarray(value, np.float32)
    wqT = np.ascontiguousarray(np.asarray(Wq, np.float32).T.astype(ml_dtypes.bfloat16))
    wkT = np.ascontiguousarray(np.asarray(Wk, np.float32).T.astype(ml_dtypes.bfloat16))
    wvT = np.ascontiguousarray(np.asarray(Wv, np.float32).T.astype(ml_dtypes.bfloat16))
    woT = np.ascontiguousarray(np.asarray(Wo, np.float32).T.astype(ml_dtypes.bfloat16))
    bo_r = np.ascontiguousarray(np.asarray(bo, np.float32).reshape(KT, 128, 1))
    in_maps = []
    for c in range(NC):
        b, h = c // 2, c % 2
        s0 = h * SC
        xqT = np.ascontiguousarray(
            q[s0 : s0 + SC, b, :].T.astype(ml_dtypes.bfloat16)
        )
        kh = np.zeros((SH, E), np.float32)
        vh = np.zeros((SH, E), np.float32)
        lo, hi = s0 - W, s0 + SC + W
        glo, ghi = max(lo, 0), min(hi, S)
        kh[glo - lo : ghi - lo] = k[glo:ghi, b, :]
        vh[glo - lo : ghi - lo] = v[glo:ghi, b, :]
        xkT = np.ascontiguousarray(kh.T.astype(ml_dtypes.bfloat16))
        xvT = np.ascontiguousarray(vh.T.astype(ml_dtypes.bfloat16))
        mLc = np.full((128, 1), 0.0 if h == 0 else 1.0, np.float32)
        mRc = np.full((128, 1), 0.0 if h == 1 else 1.0, np.float32)
        in_maps.append(
            {
                "xq": xqT, "xk": xkT, "xv": xvT,
                "wq": wqT, "wk": wkT, "wv": wvT, "wo": woT,
                "bo": bo_r, "maskL": mLc, "maskR": mRc,
            }
        )
    return in_maps


def kernel(**inputs) -> np.ndarray:
    if "nc" not in _COMPILED:
        _COMPILED["nc"] = _build_nc()
    nc = _COMPILED["nc"]
    in_maps = _shard_inputs(**inputs)
    res = run_bass_kernel_spmd(nc, in_maps, list(range(NC)))
    out = np.empty((S, B, E), np.float32)
    for c in range(NC):
        b, h = c // 2, c % 2
        s0 = h * SC
        out[s0 : s0 + SC, b, :] = res.results[c]["out"].T
    return out


# revision 8
# speedup vs baseline: 2.2502x; 1.3162x over previous
"""Local multi-head attention (window=128, look 1/1) on 8 trn2 cores.

Sharding: core c owns (batch b = c//2, sequence half h = c%2) = 1024 query
positions; k/v are recomputed over a 1280-position halo slice (one extra
128-window each side, zero-padded at the global sequence edges).

Attention runs in transposed-score orientation to avoid PE transposes:
  scoresT[j, i] = k_j . q_i  per (head, key-window u), with the query block
  being the up-to-3 query windows that attend key-window u. exp() runs on
  the scalar engine (PSUM -> SBUF bf16); softmax denominators come from
  col-tiled ones-matmuls into the same partition rows as the output
  accumulator, so the normalize is two aligned vector ops
  (reciprocal_approx_fast + multiply). Zero-padded edge key-windows would
  contribute exp(0)=1 per fake key, so their exp-scores are multiplied by
  a per-core 0/1 mask (reproducing the reference's -inf edge masking).

Head pairs (2t, 2t+1) share feature tile t: scores matmuls are row-tiled
(K=64 at array rows 0:64 / 64:128), attention@V and denominator matmuls
are col-tiled (M=64 writing partitions 0:64 / 64:128), so both heads run
concurrently in the PE array. Scores+exp for pair t are interleaved into
v-projection chunk t so the scalar-engine exp stream overlaps PE work.
"""

import sys

sys.path.insert(0, "/opt/trn_rl_repo")

import ml_dtypes
import numpy as np

import concourse.bass as bass
import concourse.mybir as mybir
from concourse import bacc
from concourse.bass_utils import run_bass_kernel_spmd
from concourse.tile import TileContext

S, B, E, H, HD, W = 2048, 4, 1024, 16, 64, 128
NC = 8
SC = S // 2           # 1024 positions per core (one batch, half the sequence)
SH = SC + 2 * W       # 1280 halo positions = 10 key windows
NW = SC // W          # 8 owned query windows
NKW = SH // W         # 10 key windows (halo coords); owned = 1..8
KT = E // 128         # 8 contraction tiles
F32 = mybir.dt.float32
BF16 = mybir.dt.bfloat16

_COMPILED = {}


def _qblock(u):
    """Query-window range (halo coords, clipped to owned 1..NW) attending
    key-window u, as column range [qc0, qc1) of the 1024 owned queries."""
    wlo, whi = max(u - 1, 1), min(u + 1, NW)
    return (wlo - 1) * W, whi * W


def _build_nc():
    nc = bacc.Bacc("TRN2", target_bir_lowering=False, debug=False, num_devices=NC)
    xq = nc.dram_tensor("xq", [E, SC], BF16, kind="ExternalInput").ap()
    xk = nc.dram_tensor("xk", [E, SH], BF16, kind="ExternalInput").ap()
    xv = nc.dram_tensor("xv", [E, SH], BF16, kind="ExternalInput").ap()
    wq = nc.dram_tensor("wq", [E, E], BF16, kind="ExternalInput").ap()
    wk = nc.dram_tensor("wk", [E, E], BF16, kind="ExternalInput").ap()
    wv = nc.dram_tensor("wv", [E, E], BF16, kind="ExternalInput").ap()
    wo = nc.dram_tensor("wo", [E, E], BF16, kind="ExternalInput").ap()
    bo = nc.dram_tensor("bo", [KT, 128, 1], F32, kind="ExternalInput").ap()
    maskL = nc.dram_tensor("maskL", [128, 1], F32, kind="ExternalInput").ap()
    maskR = nc.dram_tensor("maskR", [128, 1], F32, kind="ExternalInput").ap()
    out = nc.dram_tensor("out", [E, SC], F32, kind="ExternalOutput").ap()

    with TileContext(nc) as tc:
        with (
            tc.tile_pool(name="persist", bufs=1) as pp,
            tc.tile_pool(name="wx", bufs=2) as wx,
            tc.tile_pool(name="ppj", bufs=2, space="PSUM") as ppj,
            tc.tile_pool(name="scp", bufs=2, space="PSUM") as scp,
            tc.tile_pool(name="pop", bufs=2, space="PSUM") as pop,
            tc.tile_pool(name="dnp", bufs=2, space="PSUM") as dnp,
            tc.tile_pool(name="esc", bufs=24) as esc,
            tc.tile_pool(name="rcp", bufs=2) as rcp,
            tc.tile_pool(name="ot", bufs=2) as otp,
        ):
            def load_w(w_dram):
                w_sb = [wx.tile([128, E], BF16, name=f"w{k}", tag=f"w{k}") for k in range(KT)]
                for k in range(KT):
                    nc.sync.dma_start(out=w_sb[k][:], in_=w_dram[bass.ts(k, 128), :])
                return w_sb

            def load_x(x_dram, n_cols):
                # two column-chunks per tile so the first matmuls start
                # before the whole input lands
                x_sb = [wx.tile([128, SH], BF16, name=f"x{k}", tag=f"x{k}") for k in range(KT)]
                for k in range(KT):
                    nc.sync.dma_start(out=x_sb[k][:, 0:512], in_=x_dram[bass.ts(k, 128), 0:512])
                for k in range(KT):
                    nc.sync.dma_start(
                        out=x_sb[k][:, 512:n_cols], in_=x_dram[bass.ts(k, 128), 512:n_cols]
                    )
                return x_sb

            # ---- k projection (feature-major: kT[f_tile][128, 1280]) ----
            w_sb = load_w(wk)
            x_sb = load_x(xk, SH)

            qT = [pp.tile([128, SC], BF16, name=f"qT{m}", tag=f"qT{m}") for m in range(KT)]
            kT = [pp.tile([128, SH], BF16, name=f"kT{m}", tag=f"kT{m}") for m in range(KT)]
            vb = [pp.tile([128, E], BF16, name=f"v{m}", tag=f"v{m}") for m in range(NKW)]
            aoT = [pp.tile([128, SC], BF16, name=f"aoT{m}", tag=f"aoT{m}") for m in range(KT)]
            bo_sb = [pp.tile([128, 1], F32, name=f"bo{g}", tag=f"bo{g}") for g in range(KT)]
            for g in range(KT):
                nc.sync.dma_start(out=bo_sb[g][:], in_=bo[g])
            mL = pp.tile([128, 1], F32, name="mL", tag="mL")
            mR = pp.tile([128, 1], F32, name="mR", tag="mR")
            nc.sync.dma_start(out=mL[:], in_=maskL)
            nc.sync.dma_start(out=mR[:], in_=maskR)
            ones = pp.tile([128, 64], BF16, name="ones", tag="ones")
            nc.vector.memset(ones[:], 1.0)

            for m in range(KT):
                for i, (c0, c1) in enumerate(((0, 512), (512, 1024), (1024, 1280))):
                    ps = ppj.tile([128, 512], F32, name="ps", tag="ps")
                    for k in range(KT):
                        nc.tensor.matmul(
                            ps[:, 0 : c1 - c0],
                            w_sb[k][:, bass.ts(m, 128)],
                            x_sb[k][:, c0:c1],
                            start=(k == 0),
                            stop=(k == KT - 1),
                        )
                    if (m + i) % 2:
                        nc.vector.tensor_copy(kT[m][:, c0:c1], ps[:, 0 : c1 - c0])
                    else:
                        nc.scalar.copy(kT[m][:, c0:c1], ps[:, 0 : c1 - c0])

            # ---- q projection (feature-major: qT[f_tile][128, 1024]) ----
            w_sb = load_w(wq)
            x_sb = load_x(xq, SC)
            for m in range(KT):
                for n in range(2):
                    ps = ppj.tile([128, 512], F32, name="ps", tag="ps")
                    for k in range(KT):
                        nc.tensor.matmul(
                            ps[:],
                            w_sb[k][:, bass.ts(m, 128)],
                            x_sb[k][:, bass.ts(n, 512)],
                            start=(k == 0),
                            stop=(k == KT - 1),
                        )
                    if (m + n) % 2:
                        nc.vector.tensor_copy(qT[m][:, bass.ts(n, 512)], ps[:])
                    else:
                        nc.scalar.copy(qT[m][:, bass.ts(n, 512)], ps[:])

            # ---- v projection (position-major) + interleaved scores/exp ----
            w_sb = load_w(wv)
            x_sb = load_x(xv, SH)
            et = [{} for _ in range(KT)]

            def scores_block(t):
                for u in range(NKW):
                    qc0, qc1 = _qblock(u)
                    n = qc1 - qc0
                    eu = []
                    for hh, r0 in ((0, 0), (1, 64)):
                        sc = scp.tile([128, 384], F32, name="sc", tag="sc")
                        nc.tensor.matmul(
                            sc[:, 0:n],
                            kT[t][r0 : r0 + 64, bass.ts(u, 128)],
                            qT[t][r0 : r0 + 64, qc0:qc1],
                            start=True,
                            stop=True,
                        )
                        ee = esc.tile([128, 384], BF16, name="ee", tag="ee")
                        nc.scalar.activation(
                            ee[:, 0:n],
                            sc[:, 0:n],
                            mybir.ActivationFunctionType.Exp,
                            scale=float(HD) ** -0.5,
                        )
                        if u == 0:
                            nc.vector.tensor_scalar_mul(ee[:, 0:n], ee[:, 0:n], mL[:])
                        elif u == NKW - 1:
                            nc.vector.tensor_scalar_mul(ee[:, 0:n], ee[:, 0:n], mR[:])
                        eu.append(ee)
                    et[t][u] = (eu[0], eu[1], qc0, qc1)

            for m in range(NKW):
                for n in range(2):
                    ps = ppj.tile([128, 512], F32, name="ps", tag="ps")
                    for k in range(KT):
                        nc.tensor.matmul(
                            ps[:],
                            x_sb[k][:, bass.ts(m, 128)],
                            w_sb[k][:, bass.ts(n, 512)],
                            start=(k == 0),
                            stop=(k == KT - 1),
                        )
                    nc.vector.tensor_copy(vb[m][:, bass.ts(n, 512)], ps[:])
                if m < KT:
                    scores_block(m)

            # prefetch wo during attention
            wo_sb = load_w(wo)

            # ---- attention: A@V + denominator + normalize ----
            for t in range(KT):
                for qh in range(2):
                    g0, g1 = qh * 512, qh * 512 + 512
                    us = [u for u in range(NKW) if et[t][u][2] < g1 and et[t][u][3] > g0]
                    po = pop.tile([128, 512], F32, name="po", tag="po")
                    den = dnp.tile([128, 512], F32, name="den", tag="den")
                    for i, u in enumerate(us):
                        eA, eB, qc0, qc1 = et[t][u]
                        c0, c1 = max(qc0, g0), min(qc1, g1)
                        start, stop = (i == 0), (i == len(us) - 1)
                        for hh, (r0, ee) in enumerate(((0, eA), (64, eB))):
                            rhs = ee[:, c0 - qc0 : c1 - qc0]
                            nc.tensor.matmul(
                                po[r0 : r0 + 64, c0 - g0 : c1 - g0],
                                vb[u][:, (2 * t + hh) * 64 : (2 * t + hh + 1) * 64],
                                rhs,
                                start=start,
                                stop=stop,
                            )
                            nc.tensor.matmul(
                                den[r0 : r0 + 64, c0 - g0 : c1 - g0],
                                ones[:, 0:64],
                                rhs,
                                start=start,
                                stop=stop,
                            )
                    rec = rcp.tile([128, 512], F32, name="rec", tag="rec")
                    nc.vector.reciprocal_approx_fast(out=rec[:], in_=den[:])
                    nc.vector.tensor_mul(aoT[t][:, g0:g1], po[:], rec[:])

            # ---- output projection + bias ----
            for m in range(KT):
                ot = otp.tile([128, SC], F32, name="ot", tag="ot")
                for n in range(2):
                    ps = ppj.tile([128, 512], F32, name="ps", tag="ps")
                    for k in range(KT):
                        nc.tensor.matmul(
                            ps[:],
                            wo_sb[k][:, bass.ts(m, 128)],
                            aoT[k][:, bass.ts(n, 512)],
                            start=(k == 0),
                            stop=(k == KT - 1),
                        )
                    nc.scalar.activation(
                        ot[:, bass.ts(n, 512)],
                        ps[:],
                        mybir.ActivationFunctionType.Identity,
                        bias=bo_sb[m][:],
                    )
                nc.sync.dma_start(out=out[bass.ts(m, 128), :], in_=ot[:])

    nc.finalize()
    return nc


def _shard_inputs(query, key, value, Wq, Wk, Wv, Wo, bo, key_padding_mask):
    del key_padding_mask  # all-False in this problem; exact by construction
    q = np.asarray(query, np.float32)
    k = np.asarray(key, np.float32)
    v = np.asarray(value, np.float32)
    wqT = np.ascontiguousarray(np.asarray(Wq, np.float32).T.astype(ml_dtypes.bfloat16))
    wkT = np.ascontiguousarray(np.asarray(Wk, np.float32).T.astype(ml_dtypes.bfloat16))
    wvT = np.ascontiguousarray(np.asarray(Wv, np.float32).T.astype(ml_dtypes.bfloat16))
    woT = np.ascontiguousarray(np.asarray(Wo, np.float32).T.astype(ml_dtypes.bfloat16))
    bo_r = np.ascontiguousarray(np.asarray(bo, np.float32).reshape(KT, 128, 1))
    in_maps = []
    for c in range(NC):
        b, h = c // 2, c % 2
        s0 = h * SC
        xqT = np.ascontiguousarray(q[s0 : s0 + SC, b, :].T.astype(ml_dtypes.bfloat16))
        kh = np.zeros((SH, E), np.float32)
        vh = np.zeros((SH, E), np.float32)
        lo, hi = s0 - W, s0 + SC + W
        glo, ghi = max(lo, 0), min(hi, S)
        kh[glo - lo : ghi - lo] = k[glo:ghi, b, :]
        vh[glo - lo : ghi - lo] = v[glo:ghi, b, :]
        xkT = np.ascontiguousarray(kh.T.astype(ml_dtypes.bfloat16))
        xvT = np.ascontiguousarray(vh.T.astype(ml_dtypes.bfloat16))
        mLc = np.full((128, 1), 0.0 if h == 0 else 1.0, np.float32)
        mRc = np.full((128, 1), 0.0 if h == 1 else 1.0, np.float32)
        in_maps.append(
            {
                "xq": xqT, "xk": xkT, "xv": xvT,
                "wq": wqT, "wk": wkT, "wv": wvT, "wo": woT,
                "bo": bo_r, "maskL": mLc, "maskR": mRc,
            }
        )
    return in_maps


def kernel(**inputs) -> np.ndarray:
    if "nc" not in _COMPILED:
        _COMPILED["nc"] = _build_nc()
    nc = _COMPILED["nc"]
    in_maps = _shard_inputs(**inputs)
    res = run_bass_kernel_spmd(nc, in_maps, list(range(NC)))
    out = np.empty((S, B, E), np.float32)
    for c in range(NC):
        b, h = c // 2, c % 2
        s0 = h * SC
        out[s0 : s0 + SC, b, :] = res.results[c]["out"].T
    return out
